# revision 49
# baseline (speedup 1.0000x reference)
"""Trainium2 Bass kernel for windowed (sparse) gated attention.

Problem (hardcoded): B=2, S=4096, D=128, DI=1024 (8 heads x 128), W=128.
For each query window i (of 32), keys/values come from windows i-1,i,i+1
(3W=384 keys, zero-padded at sequence edges), plus an additive [S,S] bias
read only on those diagonal bands; softmax; gated by sigmoid(x@Wg.T+bg);
output projection Wo.

Sharding: sequence-parallel. Core c owns query windows [4c, 4c+4) for both
batches / all heads; it receives a halo'd, pre-transposed slice of seq and
the tight per-(key-window, query-window) bias blocks it needs (fp16, with
-60000 on globally-invalid key windows), so there is no inter-core
communication. Output is returned transposed per core ([B, D, 512]) and
re-assembled on the host.

Device-side layout: scores are computed transposed, simT[key, q] =
biasT + kT.T @ qT, per key-window J in -1..4 over only the valid query
band (|J - w| <= 1), in fp16 (1 cyc/row on PE at any width). Key windows
are processed in pairs of equal band size ((-1,4),(0,3),(1,2)) sharing a
2-bank PSUM tile so one Exp activation per pair moves probs to a tight
12-block fp16 slab. Softmax denominators ride a [128,2] ones-selector
stationary so two heads accumulate their column sums into one [2,512]
PSUM tile (partitions 0/1); the reciprocal is taken straight from PSUM,
broadcast across partitions by a rank-1 PE matmul, and applied to the
gated output (divides commute with AV / Wo within a head).
"""

import numpy as np

import concourse.bass as bass
import concourse.mybir as mybir
import concourse.tile as tile
from concourse import bacc

F32 = mybir.dt.float32
F32R = mybir.dt.float32r
F16 = mybir.dt.float16

B, S, D, DI, W, H, DH = 2, 4096, 128, 1024, 128, 8, 128
NCORES = 8
NWIN = S // W                 # 32 windows total
NW = NWIN // NCORES           # 4 query windows per core
SC = NW * W                   # 512 query positions per core
NJ = NW + 2                   # 6 key windows per core (with halo)
SL = NJ * W                   # 768 key positions per core
NEG = -60000.0                # fp16-safe "-inf" for bias masking

CFG = dict(nrep=1)

# key-window pairs of equal valid-band size; slab layout is pair-major.
# Widest pair first: its Exp is the longest and AV blocks for w=1,2 only
# need the first two exps, so the PE can start AV before the last exp.
PAIRS = [(1, 2), (0, 3), (-1, 4)]
WSTART = {-1: 0, 0: 0, 1: 0, 2: 1, 3: 2, 4: 3}
NK = {-1: 1, 0: 2, 1: 3, 2: 3, 3: 2, 4: 1}
BSTART = {}
_off = 0
for _Ja, _Jb in PAIRS:
    BSTART[_Ja] = _off
    _off += NK[_Ja]
    BSTART[_Jb] = _off
    _off += NK[_Jb]
NBLK = _off                   # 12 valid (key-window, q-window) blocks
DEBUG = False


def _blk(J, w):
    return BSTART[J] + w - WSTART[J]


# ---------------------------------------------------------------- device
def _build_device(nc, t):
    AF = mybir.ActivationFunctionType
    ALU = mybir.AluOpType

    from contextlib import ExitStack

    with tile.TileContext(nc) as tc, ExitStack() as st:
        cpool = st.enter_context(tc.tile_pool(name="consts", bufs=1))
        wpool = st.enter_context(tc.tile_pool(name="weights", bufs=1))
        bpool = st.enter_context(tc.tile_pool(name="batch", bufs=2))
        apool = st.enter_context(tc.tile_pool(name="attn", bufs=4))
        opool = st.enter_context(tc.tile_pool(name="og", bufs=1))
        ypool = st.enter_context(tc.tile_pool(name="yout", bufs=2))
        ps3 = st.enter_context(tc.tile_pool(name="ps3", bufs=2, space="PSUM"))
        psA = st.enter_context(tc.tile_pool(name="psA", bufs=1, space="PSUM"))
        psS = st.enter_context(tc.tile_pool(name="psS", bufs=1, space="PSUM"))
        psB = st.enter_context(tc.tile_pool(name="psB", bufs=2, space="PSUM"))

        # ---- inputs/weights in bus-priority order: the first PE work
        # (q/k proj of head 0, then scores) gates the pipeline start
        wq = wpool.tile([128, DI], F16, tag="wq")
        nc.sync.dma_start(wq, t["wqT"][:])
        x0 = bpool.tile([128, SL], F16, tag="x", name="x_0")
        nc.sync.dma_start(x0[:, 0:640], t["xT"][0][:, 0:640])
        wk = wpool.tile([128, DI], F16, tag="wk")
        nc.scalar.dma_start(wk, t["wkT"][:])
        nc.sync.dma_start(x0[:, 640:SL], t["xT"][0][:, 640:SL])
        # all small constants ride ONE DMA (HWDGE descriptor generation is
        # a single serialized ~630ns/DMA resource): ident, ones2 selector,
        # bq8/bg8, and the [2,2,128] recip row-selector (partitions 0:2)
        cb = cpool.tile([128, 340], F32, tag="cb")
        nc.scalar.dma_start(cb, t["cblob"][:])
        wv = wpool.tile([128, DI], F16, tag="wv")
        nc.scalar.dma_start(wv, t["wvT"][:])
        ident = cb[:, 0:64].bitcast(F16)
        ones2 = cb[:, 64:66].bitcast(F16).rearrange("p (a b) -> p a b", a=2)
        bq8 = cb[:, 66:74]
        bg8 = cb[:, 74:82]
        sel2 = cb[0:2, 84:212].bitcast(F16) \
            .rearrange("p (a b) -> p a b", a=2)
        # wg+wo in one DMA too (both first needed well after startup)
        wgo = wpool.tile([128, DI + 1024], F16, tag="wgo")
        nc.sync.dma_start(wgo, t["wgoT"][:])
        wg = wgo[:, 0:DI]
        wo = wgo[:, DI : DI + 1024].rearrange("p (a b) -> p a b", a=8)

        def prologue(b, rep, x=None):
            """Allocate batch tiles, DMA inputs, v-projection, proj(0)."""
            st_ = dict(b=b, rep=rep, pending=None, attnP=None)
            if x is None:
                x = bpool.tile([128, SL], F16, tag="x", name=f"x_{b}_{rep}")
                nc.sync.dma_start(x, t["xT"][b])
            biasF = bpool.tile([128, NBLK, 128], F16, tag="bias",
                               name=f"bias_{b}_{rep}")
            nc.scalar.dma_start(biasF, t["biasT"][b])
            st_["x"], st_["biasF"] = x, biasF
            st_["og"] = opool.tile([128, H, SC], F16, tag=f"og{b}",
                                   name=f"og{b}_{rep}")
            st_["qT"] = bpool.tile([128, H, SC], F16, tag="qT",
                                   name=f"qT_{b}_{rep}")
            st_["gT"] = bpool.tile([128, H, SC], F32, tag="gT",
                                   name=f"gT_{b}_{rep}")
            st_["kT"] = bpool.tile([128, H, SL], F16, tag="kT",
                                   name=f"kT_{b}_{rep}")
            vv = bpool.tile([128, NJ, DI], F16, tag="vv",
                            name=f"vv_{b}_{rep}")
            st_["vv"] = vv
            # minimal prologue: q/k for heads 0/1 and the first two v
            # chunks; v chunks 2-5 are deferred into head 0 so its scores
            # start as early as possible
            proj_qk(st_, 0)
            pv_emit(st_, 0)
            qadd_flush(st_)
            pv_emit(st_, 1)
            proj_g(st_, 0)
            proj_qk(st_, 1)
            qadd_flush(st_)
            return st_

        def pv_emit(st_, sc_i):
            x, vv = st_["x"], st_["vv"]
            xs = x[:, sc_i * 128 : (sc_i + 1) * 128]
            pv = ps3.tile([128, 2, 512], F32, tag="ps3")
            nc.tensor.matmul(pv[:, 0, :], xs, wv[:, 0:512],
                             start=True, stop=True)
            nc.tensor.matmul(pv[:, 1, :], xs, wv[:, 512:1024],
                             start=True, stop=True)
            nc.vector.tensor_copy(vv[:, sc_i, 0:512], pv[:, 0, :])
            nc.scalar.copy(vv[:, sc_i, 512:1024], pv[:, 1, :])

        def proj_qk(st_, c):
            # q/k projections for head-chunk c (pg emitted separately:
            # its psA slot reuse would head-of-line-block the PE queue
            # while the DVE drains the qT add)
            x, xc = st_["x"], st_["x"][:, W : W + SC]
            pq = psA.tile([128, 512], F32, tag="psA")
            nc.tensor.matmul(pq, wq[:, c * 128 : (c + 1) * 128], xc,
                             start=True, stop=True)
            st_["qadd"] = (c, pq)
            pk = ps3.tile([128, 2, 512], F32, tag="ps3")
            nc.tensor.matmul(pk[:, 0, :], wk[:, c * 128 : (c + 1) * 128],
                             x[:, 0:512], start=True, stop=True)
            nc.tensor.matmul(pk[:, 1, 0:256], wk[:, c * 128 : (c + 1) * 128],
                             x[:, 512:768], start=True, stop=True)
            pkf = pk.rearrange("p a b -> p (a b)")[:, 0:SL]
            nc.vector.tensor_copy(st_["kT"][:, c, :], pkf)

        def qadd_flush(st_):
            if st_.get("qadd") is not None:
                c, pq = st_.pop("qadd")
                nc.vector.tensor_scalar_add(st_["qT"][:, c, :], pq,
                                            bq8[:, c : c + 1])

        def proj_g(st_, c):
            xc = st_["x"][:, W : W + SC]
            pg = psA.tile([128, 512], F32, tag="psA")
            nc.tensor.matmul(pg, wg[:, c * 128 : (c + 1) * 128], xc,
                             start=True, stop=True)
            # sigmoid(z) = 0.5*tanh(0.5 z)+0.5; +1 folded into gating,
            # *0.5 into Wo (host-folded)
            nc.scalar.activation(st_["gT"][:, c, :], pg, AF.Tanh,
                                 bias=bg8[:, c : c + 1], scale=0.5)

        def pair_tail(st_):
            # recip broadcast + normalization for head pair i; deferred so
            # the PE work in between hides the DVE reciprocal latency
            i, rb = st_["pending"]
            st_["pending"] = None
            prb = ps3.tile([128, 2, 512], F32, tag="ps3")
            nc.tensor.matmul(prb[:, 0, :], sel2[:, 0, :], rb,
                             start=True, stop=True)
            nc.tensor.matmul(prb[:, 1, :], sel2[:, 1, :], rb,
                             start=True, stop=True)
            ogp = st_["og"][:, 2 * i : 2 * i + 2, :] \
                .rearrange("p a b -> p (a b)")
            nc.vector.tensor_tensor(
                ogp, ogp, prb.rearrange("p a b -> p (a b)"), ALU.mult)

        def head(st_, h):
            b, rep = st_["b"], st_["rep"]
            qT, kT, vv, gT = st_["qT"], st_["kT"], st_["vv"], st_["gT"]
            biasF, og = st_["biasF"], st_["og"]
            attnT = apool.tile([128, NBLK, 128], F16, tag="attnT")
            for g2, (Ja, Jb) in enumerate(PAIRS):
                nk = NK[Ja]
                if nk == 1:
                    # the (-1,4) pair needs only 1KB: park it in a psB
                    # slot so the ps3 rotation never blocks on an exp
                    psim = psB.tile([128, 2, 128], F32, tag="psB",
                                    name=f"psim2_{b}_{h}_{rep}")
                else:
                    psim = ps3.tile([128, 2, 512], F32, tag="ps3")
                for j, J in enumerate((Ja, Jb)):
                    o = WSTART[J]
                    out = psim[:, j, 0 : nk * 128]
                    nc.tensor.matmul(
                        out, ident,
                        biasF[:, BSTART[J] : BSTART[J] + nk, :]
                        .rearrange("p a b -> p (a b)"),
                        start=True, stop=False)
                    nc.tensor.matmul(
                        out, kT[:, h, (J + 1) * 128 : (J + 2) * 128],
                        qT[:, h, o * 128 : (o + nk) * 128],
                        start=False, stop=True)
                slab = attnT[:, BSTART[Ja] : BSTART[Ja] + 2 * nk, :]
                nc.scalar.activation(
                    slab.rearrange("p (j w) e -> p j (w e)", j=2),
                    psim[:, :, 0 : nk * 128], AF.Exp)

            if h == 0:
                for sc_i in range(2, NJ):
                    pv_emit(st_, sc_i)
            if h + 2 < H:
                proj_qk(st_, h + 2)
            if st_["pending"] is not None:
                pair_tail(st_)

            # AV: one strictly-sequential 3-matmul group per w-column
            # region (interleaved opens in one PSUM bank are illegal);
            # w order follows exp availability: w1,w2 need only the
            # first two exps, w0,w3 also the last. On odd heads the
            # pair's column sums go FIRST so the reciprocal chain starts
            # as early as possible, covered by the AV matmuls behind it.
            def sums_emit():
                psums = psS.tile([2, 512], F32, tag="psS",
                                 name=f"psums_{b}_{h}_{rep}")
                for w in (1, 2, 0, 3):
                    for hp in (0, 1):
                        at = attnT if hp else st_["attnP"]
                        for jj in range(3):
                            J = w + jj - 1
                            nc.tensor.matmul(
                                psums[:, w * 128 : (w + 1) * 128],
                                ones2[:, hp, :], at[:, _blk(J, w), :],
                                start=(hp == 0 and jj == 0),
                                stop=(hp == 1 and jj == 2))
                return psums

            # on the very last head the sums go first: the reciprocal
            # chain is the program's tail critical path
            psums = sums_emit() if h == H - 1 else None

            poT = psB.tile([128, 512], F32, tag="psB",
                           name=f"poT_{b}_{h}_{rep}")
            for w in (1, 2, 0, 3):
                for jj in range(3):
                    J = w + jj - 1
                    a_sl = attnT[:, _blk(J, w), :]
                    nc.tensor.matmul(
                        poT[:, w * 128 : (w + 1) * 128],
                        vv[:, J + 1, h * 128 : (h + 1) * 128], a_sl,
                        start=(jj == 0), stop=(jj == 2))

            # gate immediately (normalization applied per pair below)
            nc.vector.scalar_tensor_tensor(
                og[:, h, :], gT[:, h, :], 1.0, poT, ALU.add, ALU.mult)
            qadd_flush(st_)
            if h + 1 < H:
                proj_g(st_, h + 1)

            if h % 2 == 1:
                if psums is None:
                    psums = sums_emit()
                i = h // 2
                # fast reciprocal written straight into an f32r tile (the
                # DVE rounds on write, satisfying the f32r-matmul input
                # rule without a separate copy)
                from concourse.dve_ops import (RECIP_APPROX_FAST_CONSTS,
                                               RECIPROCAL_APPROX_FAST)
                rb2 = bpool.tile([2, 512], F16, tag=f"rb2_{i}",
                                 name=f"rb2_{b}_{i}_{rep}")
                c_ = RECIP_APPROX_FAST_CONSTS
                with nc.allow_low_precision(reason="softmax recip"):
                    nc.vector._custom_dve(
                        RECIPROCAL_APPROX_FAST, out=rb2, in0=psums,
                        s0=c_["s0"], s1=c_["s1"], imm2=c_["imm2"])
                if DEBUG and b == 0 and i == 0:
                    pass  # d_sums debug dropped (rb2 is fp16 now)
                st_["pending"] = (i, rb2)
            st_["attnP"] = attnT

            if DEBUG and b == 0 and h == 0:
                nc.sync.dma_start(t["d_attnT"][:], attnT[:])
                po_sb = bpool.tile([128, 512], F32, tag="po_sb")
                nc.vector.tensor_copy(po_sb, poT)
                nc.sync.dma_start(t["d_oT"][:], po_sb)

        def outproj(st_):
            b, og = st_["b"], st_["og"]
            if DEBUG and b == 0:
                for nm, tl in [("d_qT", st_["qT"]), ("d_gT", st_["gT"]),
                               ("d_kT", st_["kT"]), ("d_vv", st_["vv"]),
                               ("d_og", og)]:
                    nc.sync.dma_start(t[nm][:], tl[:])
            # chunks 0-5 first so they hide the last pair's reciprocal;
            # the last pair is then normalized and folded in per-head, and
            # y is copied/DMA'd in halves to overlap the final transfer
            i, rb = st_["pending"]
            st_["pending"] = None
            pf = psA.tile([128, 512], F32, tag="psA")
            for c in range(6):
                nc.tensor.matmul(pf, wo[:, c, :], og[:, c, :],
                                 start=(c == 0), stop=False)
            prb = ps3.tile([128, 2, 512], F32, tag="ps3")
            for j, c in enumerate((6, 7)):
                nc.tensor.matmul(prb[:, j, :], sel2[:, j, :], rb,
                                 start=True, stop=True)
                nc.vector.tensor_tensor(og[:, c, :], og[:, c, :],
                                        prb[:, j, :], ALU.mult)
                nc.tensor.matmul(pf, wo[:, c, :], og[:, c, :],
                                 start=False, stop=(c == 7))
            y = ypool.tile([128, 512], F16, tag="y")
            nc.scalar.copy(y[:, 0:256], pf[:, 0:256])
            nc.sync.dma_start(t["yT"][b][:, 0:256], y[:, 0:256])
            nc.vector.tensor_copy(y[:, 256:512], pf[:, 256:512])
            nc.scalar.dma_start(t["yT"][b][:, 256:512], y[:, 256:512])

        # software pipeline: the two batches are interleaved head-by-head
        # with a 2-head stagger, so every intra-head latency chain (exp ->
        # AV, gating -> PSUM reuse, reciprocal -> normalize) is covered by
        # the other batch's independent matmuls
        st0 = prologue(0, 0, x=x0)
        for rep in range(CFG["nrep"]):
            head(st0, 0)
            st1 = prologue(1, rep)
            for k in range(1, H):
                head(st0, k)
                head(st1, k - 1)
            nxt = None
            if rep + 1 < CFG["nrep"]:
                nxt = prologue(0, rep + 1)
            outproj(st0)
            head(st1, H - 1)
            outproj(st1)
            if nxt is not None:
                st0 = nxt


# ---------------------------------------------------------------- build
_CACHE = {}


def _get_nc():
    key = tuple(sorted(CFG.items()))
    if _CACHE.get("key") == key:
        return _CACHE["nc"], _CACHE["t"]
    nc = bacc.Bacc(None, target_bir_lowering=False)
    t = dict(
        xT=nc.dram_tensor("xT", [B, 128, SL], F16, kind="ExternalInput"),
        biasT=nc.dram_tensor("biasT", [B, 128, NBLK, 128], F16,
                             kind="ExternalInput"),
        wqT=nc.dram_tensor("wqT", [128, DI], F16, kind="ExternalInput"),
        wkT=nc.dram_tensor("wkT", [128, DI], F16, kind="ExternalInput"),
        wvT=nc.dram_tensor("wvT", [128, DI], F16, kind="ExternalInput"),
        wgoT=nc.dram_tensor("wgoT", [128, DI + 1024], F16,
                            kind="ExternalInput"),
        cblob=nc.dram_tensor("cblob", [128, 340], F32,
                             kind="ExternalInput"),
        yT=nc.dram_tensor("yT", [B, 128, SC], F16, kind="ExternalOutput"),
    )
    if DEBUG:
        for nm, shp, dt_ in [("d_qT", [128, H, SC], F16),
                             ("d_gT", [128, H, SC], F32),
                             ("d_kT", [128, H, SL], F16),
                             ("d_vv", [128, NJ, DI], F16),
                             ("d_og", [128, H, SC], F16),
                             ("d_attnT", [128, NBLK, 128], F16),
                             ("d_oT", [128, 512], F32),
                             ("d_sums", [2, 512], F32)]:
            t[nm] = nc.dram_tensor(nm, shp, dt_, kind="ExternalOutput")
    _build_device(nc, t)
    nc.compile()
    _CACHE["nc"], _CACHE["t"], _CACHE["key"] = nc, t, key
    return nc, t


# ---------------------------------------------------------------- host
def _prep_shared(Wq, bq, Wkv, Wg, bg, Wo):
    scale = DH ** -0.5
    wqT = np.ascontiguousarray((Wq * scale).T, np.float16)          # [128,1024]
    wkT = np.ascontiguousarray(Wkv[:DI].T, np.float16)
    wvT = np.ascontiguousarray(Wkv[DI:].T, np.float16)
    wgT = np.ascontiguousarray(Wg.T, np.float16)
    # gating uses (tanh(0.5 z)+1) and final matmul absorbs the 0.5
    woT = np.ascontiguousarray(
        (0.5 * Wo).T.reshape(8, 128, 128).transpose(1, 0, 2), np.float16
    )                                                                # [128,8,128]
    bq8 = np.ascontiguousarray((bq * scale).reshape(8, 128).T, np.float32)
    bg8 = np.ascontiguousarray((bg * 0.5).reshape(8, 128).T, np.float32)
    ident = np.eye(128, dtype=np.float16)
    ones2 = np.zeros((128, 2, 2), np.float16)
    ones2[:, 0, 0] = 1.0
    ones2[:, 1, 1] = 1.0
    sel2 = np.zeros((2, 2, 128), np.float32)
    sel2[0, 0, :] = 1.0
    sel2[1, 1, :] = 1.0
    cblob = np.zeros((128, 340), np.float32)
    cblob[:, 0:64] = ident.view(np.float32).reshape(128, 64)
    cblob[:, 64:66] = ones2.reshape(128, 4).view(np.float32)
    cblob[:, 66:74] = bq8
    cblob[:, 74:82] = bg8
    cblob[0:2, 84:212] = sel2.reshape(2, 256).astype(np.float16) \
        .view(np.float32)
    wgoT = np.concatenate(
        [wgT, woT.reshape(128, 1024)], axis=1)             # [128,2048] f16
    return dict(wqT=wqT, wkT=wkT, wvT=wvT, wgoT=wgoT, cblob=cblob)


def _prep_core(c, seq, attn_bias):
    lo = c * SC - W
    hi = c * SC + SC + W
    xs = np.zeros((B, SL, D), np.float16)
    a, bnd = max(lo, 0), min(hi, S)
    xs[:, a - lo : bnd - lo, :] = seq[:, a:bnd, :].astype(np.float16)
    xT = np.ascontiguousarray(xs.transpose(0, 2, 1))                 # [B,128,768]

    br = attn_bias.reshape(B, NWIN, W, NWIN, W)
    biasT = np.full((B, 128, NBLK, 128), NEG, np.float16)
    for J in range(-1, NW + 1):
        gk = NW * c + J                     # global key window
        if not (0 <= gk < NWIN):
            continue
        for wi in range(NK[J]):
            w = WSTART[J] + wi
            gq = NW * c + w
            blk = br[:, gq, :, gk, :]       # [B, q(128), k(128)]
            biasT[:, :, BSTART[J] + wi, :] = \
                blk.transpose(0, 2, 1).astype(np.float16)
    return xT, biasT


def kernel(seq, mask, attn_bias, Wq, bq, Wkv, Wg, bg, Wo):
    from concourse.bass_utils import run_bass_kernel_spmd

    nc, _ = _get_nc()
    seq = np.asarray(seq, np.float32)
    attn_bias = np.asarray(attn_bias, np.float32)
    shared = _prep_shared(
        np.asarray(Wq, np.float32), np.asarray(bq, np.float32),
        np.asarray(Wkv, np.float32), np.asarray(Wg, np.float32),
        np.asarray(bg, np.float32), np.asarray(Wo, np.float32),
    )
    in_maps = []
    for c in range(NCORES):
        xT, biasT = _prep_core(c, seq, attn_bias)
        in_maps.append(dict(xT=xT, biasT=biasT, **shared))

    res = run_bass_kernel_spmd(nc, in_maps, core_ids=list(range(NCORES)))
    y = np.empty((B, S, D), np.float32)
    for c in range(NCORES):
        yT = res.results[c]["yT"]                                    # [B,128,512]
        y[:, c * SC : (c + 1) * SC, :] = yT.transpose(0, 2, 1).astype(np.float32)
    return y


# revision 50
# speedup vs baseline: 1.0123x; 1.0123x over previous
"""Trainium2 Bass kernel for windowed (sparse) gated attention.

Problem (hardcoded): B=2, S=4096, D=128, DI=1024 (8 heads x 128), W=128.
For each query window i (of 32), keys/values come from windows i-1,i,i+1
(3W=384 keys, zero-padded at sequence edges), plus an additive [S,S] bias
read only on those diagonal bands; softmax; gated by sigmoid(x@Wg.T+bg);
output projection Wo.

Sharding: sequence-parallel. Core c owns query windows [4c, 4c+4) for both
batches / all heads; it receives a halo'd, pre-transposed slice of seq and
the tight per-(key-window, query-window) bias blocks it needs (fp16, with
-60000 on globally-invalid key windows), so there is no inter-core
communication. Output is returned transposed per core ([B, D, 512]) and
re-assembled on the host.

Device-side layout: scores are computed transposed, simT[key, q] =
biasT + kT.T @ qT, per key-window J in -1..4 over only the valid query
band (|J - w| <= 1), in fp16 (1 cyc/row on PE at any width). Key windows
are processed in pairs of equal band size ((-1,4),(0,3),(1,2)) sharing a
2-bank PSUM tile so one Exp activation per pair moves probs to a tight
12-block fp16 slab. Softmax denominators ride a [128,2] ones-selector
stationary so two heads accumulate their column sums into one [2,512]
PSUM tile (partitions 0/1); the reciprocal is taken straight from PSUM,
broadcast across partitions by a rank-1 PE matmul, and applied to the
gated output (divides commute with AV / Wo within a head).
"""

import numpy as np

import concourse.bass as bass
import concourse.mybir as mybir
import concourse.tile as tile
from concourse import bacc

F32 = mybir.dt.float32
F32R = mybir.dt.float32r
F16 = mybir.dt.float16

B, S, D, DI, W, H, DH = 2, 4096, 128, 1024, 128, 8, 128
NCORES = 8
NWIN = S // W                 # 32 windows total
NW = NWIN // NCORES           # 4 query windows per core
SC = NW * W                   # 512 query positions per core
NJ = NW + 2                   # 6 key windows per core (with halo)
SL = NJ * W                   # 768 key positions per core
NEG = -60000.0                # fp16-safe "-inf" for bias masking

CFG = dict(nrep=1)

# key-window pairs of equal valid-band size; slab layout is pair-major.
# Widest pair first: its Exp is the longest and AV blocks for w=1,2 only
# need the first two exps, so the PE can start AV before the last exp.
PAIRS = [(1, 2), (0, 3), (-1, 4)]
WSTART = {-1: 0, 0: 0, 1: 0, 2: 1, 3: 2, 4: 3}
NK = {-1: 1, 0: 2, 1: 3, 2: 3, 3: 2, 4: 1}
BSTART = {}
_off = 0
for _Ja, _Jb in PAIRS:
    BSTART[_Ja] = _off
    _off += NK[_Ja]
    BSTART[_Jb] = _off
    _off += NK[_Jb]
NBLK = _off                   # 12 valid (key-window, q-window) blocks
DEBUG = False


def _blk(J, w):
    return BSTART[J] + w - WSTART[J]


# ---------------------------------------------------------------- device
def _build_device(nc, t):
    AF = mybir.ActivationFunctionType
    ALU = mybir.AluOpType

    from contextlib import ExitStack

    with tile.TileContext(nc) as tc, ExitStack() as st:
        cpool = st.enter_context(tc.tile_pool(name="consts", bufs=1))
        wpool = st.enter_context(tc.tile_pool(name="weights", bufs=1))
        bpool = st.enter_context(tc.tile_pool(name="batch", bufs=2))
        apool = st.enter_context(tc.tile_pool(name="attn", bufs=4))
        opool = st.enter_context(tc.tile_pool(name="og", bufs=1))
        ypool = st.enter_context(tc.tile_pool(name="yout", bufs=2))
        ps3 = st.enter_context(tc.tile_pool(name="ps3", bufs=2, space="PSUM"))
        psA = st.enter_context(tc.tile_pool(name="psA", bufs=1, space="PSUM"))
        psS = st.enter_context(tc.tile_pool(name="psS", bufs=1, space="PSUM"))
        psB = st.enter_context(tc.tile_pool(name="psB", bufs=2, space="PSUM"))

        # ---- inputs/weights in bus-priority order: the first PE work
        # (q/k proj of head 0, then scores) gates the pipeline start
        wq = wpool.tile([128, DI], F16, tag="wq")
        nc.sync.dma_start(wq, t["wqT"][:])
        x0 = bpool.tile([128, SL], F16, tag="x", name="x_0")
        nc.sync.dma_start(x0[:, 0:640], t["xT"][0][:, 0:640])
        wk = wpool.tile([128, DI], F16, tag="wk")
        nc.scalar.dma_start(wk, t["wkT"][:])
        nc.sync.dma_start(x0[:, 640:SL], t["xT"][0][:, 640:SL])
        # all small constants ride ONE DMA (HWDGE descriptor generation is
        # a single serialized ~630ns/DMA resource): ident, ones2 selector,
        # bq8/bg8, and the [2,2,128] recip row-selector (partitions 0:2)
        cb = cpool.tile([128, 340], F32, tag="cb")
        nc.scalar.dma_start(cb, t["cblob"][:])
        wv = wpool.tile([128, DI], F16, tag="wv")
        nc.scalar.dma_start(wv, t["wvT"][:])
        ident = cb[:, 0:64].bitcast(F16)
        ones2 = cb[:, 64:66].bitcast(F16).rearrange("p (a b) -> p a b", a=2)
        bq8 = cb[:, 66:74]
        bg8 = cb[:, 74:82]
        sel2 = cb[0:2, 84:212].bitcast(F16) \
            .rearrange("p (a b) -> p a b", a=2)
        # wg+wo in one DMA too (both first needed well after startup)
        wgo = wpool.tile([128, DI + 1024], F16, tag="wgo")
        nc.sync.dma_start(wgo, t["wgoT"][:])
        wg = wgo[:, 0:DI]
        wo = wgo[:, DI : DI + 1024].rearrange("p (a b) -> p a b", a=8)

        def prologue(b, rep, x=None):
            """Allocate batch tiles, DMA inputs, v-projection, proj(0)."""
            st_ = dict(b=b, rep=rep, pending=None, attnP=None)
            if x is None:
                x = bpool.tile([128, SL], F16, tag="x", name=f"x_{b}_{rep}")
                nc.sync.dma_start(x, t["xT"][b])
            biasF = bpool.tile([128, NBLK, 128], F16, tag="bias",
                               name=f"bias_{b}_{rep}")
            nc.scalar.dma_start(biasF, t["biasT"][b])
            st_["x"], st_["biasF"] = x, biasF
            st_["og"] = opool.tile([128, H, SC], F16, tag=f"og{b}",
                                   name=f"og{b}_{rep}")
            st_["qT"] = bpool.tile([128, H, SC], F16, tag="qT",
                                   name=f"qT_{b}_{rep}")
            st_["gT"] = bpool.tile([128, H, SC], F32, tag="gT",
                                   name=f"gT_{b}_{rep}")
            st_["kT"] = bpool.tile([128, H, SL], F16, tag="kT",
                                   name=f"kT_{b}_{rep}")
            vv = bpool.tile([128, NJ, DI], F16, tag="vv",
                            name=f"vv_{b}_{rep}")
            st_["vv"] = vv
            # minimal prologue: q/k for heads 0/1 and the first two v
            # chunks; v chunks 2-5 are deferred into head 0 so its scores
            # start as early as possible
            proj_qk(st_, 0)
            pv_emit(st_, 0)
            qadd_flush(st_)
            pv_emit(st_, 1)
            proj_g(st_, 0)
            proj_qk(st_, 1)
            qadd_flush(st_)
            return st_

        def pv_emit(st_, sc_i):
            x, vv = st_["x"], st_["vv"]
            xs = x[:, sc_i * 128 : (sc_i + 1) * 128]
            pv = ps3.tile([128, 2, 512], F32, tag="ps3")
            nc.tensor.matmul(pv[:, 0, :], xs, wv[:, 0:512],
                             start=True, stop=True)
            nc.tensor.matmul(pv[:, 1, :], xs, wv[:, 512:1024],
                             start=True, stop=True)
            nc.vector.tensor_copy(vv[:, sc_i, 0:512], pv[:, 0, :])
            nc.scalar.copy(vv[:, sc_i, 512:1024], pv[:, 1, :])

        def proj_qk(st_, c):
            # q/k projections for head-chunk c (pg emitted separately:
            # its psA slot reuse would head-of-line-block the PE queue
            # while the DVE drains the qT add)
            x, xc = st_["x"], st_["x"][:, W : W + SC]
            pq = psA.tile([128, 512], F32, tag="psA")
            nc.tensor.matmul(pq, wq[:, c * 128 : (c + 1) * 128], xc,
                             start=True, stop=True)
            st_["qadd"] = (c, pq)
            pk = ps3.tile([128, 2, 512], F32, tag="ps3")
            nc.tensor.matmul(pk[:, 0, :], wk[:, c * 128 : (c + 1) * 128],
                             x[:, 0:512], start=True, stop=True)
            nc.tensor.matmul(pk[:, 1, 0:256], wk[:, c * 128 : (c + 1) * 128],
                             x[:, 512:768], start=True, stop=True)
            pkf = pk.rearrange("p a b -> p (a b)")[:, 0:SL]
            nc.vector.tensor_copy(st_["kT"][:, c, :], pkf)

        def qadd_flush(st_):
            if st_.get("qadd") is not None:
                c, pq = st_.pop("qadd")
                nc.vector.tensor_scalar_add(st_["qT"][:, c, :], pq,
                                            bq8[:, c : c + 1])

        def proj_g(st_, c):
            xc = st_["x"][:, W : W + SC]
            pg = psA.tile([128, 512], F32, tag="psA")
            nc.tensor.matmul(pg, wg[:, c * 128 : (c + 1) * 128], xc,
                             start=True, stop=True)
            # sigmoid(z) = 0.5*tanh(0.5 z)+0.5; +1 folded into gating,
            # *0.5 into Wo (host-folded)
            nc.scalar.activation(st_["gT"][:, c, :], pg, AF.Tanh,
                                 bias=bg8[:, c : c + 1], scale=0.5)

        def pair_tail(st_):
            # recip broadcast + normalization for head pair i; deferred so
            # the PE work in between hides the DVE reciprocal latency
            i, rb = st_["pending"]
            st_["pending"] = None
            prb = ps3.tile([128, 2, 512], F32, tag="ps3")
            nc.tensor.matmul(prb[:, 0, :], sel2[:, 0, :], rb,
                             start=True, stop=True)
            nc.tensor.matmul(prb[:, 1, :], sel2[:, 1, :], rb,
                             start=True, stop=True)
            ogp = st_["og"][:, 2 * i : 2 * i + 2, :] \
                .rearrange("p a b -> p (a b)")
            nc.vector.tensor_tensor(
                ogp, ogp, prb.rearrange("p a b -> p (a b)"), ALU.mult)

        def head(st_, h):
            b, rep = st_["b"], st_["rep"]
            qT, kT, vv, gT = st_["qT"], st_["kT"], st_["vv"], st_["gT"]
            biasF, og = st_["biasF"], st_["og"]
            attnT = apool.tile([128, NBLK, 128], F16, tag="attnT")
            for g2, (Ja, Jb) in enumerate(PAIRS):
                nk = NK[Ja]
                if nk == 1:
                    # the (-1,4) pair needs only 1KB: park it in a psB
                    # slot so the ps3 rotation never blocks on an exp
                    psim = psB.tile([128, 2, 128], F32, tag="psB",
                                    name=f"psim2_{b}_{h}_{rep}")
                else:
                    psim = ps3.tile([128, 2, 512], F32, tag="ps3")
                for j, J in enumerate((Ja, Jb)):
                    o = WSTART[J]
                    out = psim[:, j, 0 : nk * 128]
                    nc.tensor.matmul(
                        out, ident,
                        biasF[:, BSTART[J] : BSTART[J] + nk, :]
                        .rearrange("p a b -> p (a b)"),
                        start=True, stop=False)
                    nc.tensor.matmul(
                        out, kT[:, h, (J + 1) * 128 : (J + 2) * 128],
                        qT[:, h, o * 128 : (o + nk) * 128],
                        start=False, stop=True)
                slab = attnT[:, BSTART[Ja] : BSTART[Ja] + 2 * nk, :]
                nc.scalar.activation(
                    slab.rearrange("p (j w) e -> p j (w e)", j=2),
                    psim[:, :, 0 : nk * 128], AF.Exp)

            if h == 0:
                for sc_i in range(2, NJ):
                    pv_emit(st_, sc_i)
            if h + 2 < H:
                proj_qk(st_, h + 2)
            if st_["pending"] is not None:
                pair_tail(st_)

            # AV: one strictly-sequential 3-matmul group per w-column
            # region (interleaved opens in one PSUM bank are illegal);
            # w order follows exp availability: w1,w2 need only the
            # first two exps, w0,w3 also the last. On odd heads the
            # pair's column sums go FIRST so the reciprocal chain starts
            # as early as possible, covered by the AV matmuls behind it.
            def sums_emit():
                psums = psS.tile([2, 512], F32, tag="psS",
                                 name=f"psums_{b}_{h}_{rep}")
                for w in (1, 2, 0, 3):
                    for hp in (0, 1):
                        at = attnT if hp else st_["attnP"]
                        for jj in range(3):
                            J = w + jj - 1
                            nc.tensor.matmul(
                                psums[:, w * 128 : (w + 1) * 128],
                                ones2[:, hp, :], at[:, _blk(J, w), :],
                                start=(hp == 0 and jj == 0),
                                stop=(hp == 1 and jj == 2))
                return psums

            # on the very last head the sums go first: the reciprocal
            # chain is the program's tail critical path
            psums = sums_emit() if h == H - 1 else None

            poT = psB.tile([128, 512], F32, tag="psB",
                           name=f"poT_{b}_{h}_{rep}")
            for w in (1, 2, 0, 3):
                for jj in range(3):
                    J = w + jj - 1
                    a_sl = attnT[:, _blk(J, w), :]
                    nc.tensor.matmul(
                        poT[:, w * 128 : (w + 1) * 128],
                        vv[:, J + 1, h * 128 : (h + 1) * 128], a_sl,
                        start=(jj == 0), stop=(jj == 2))

            # gate immediately (normalization applied per pair below)
            nc.vector.scalar_tensor_tensor(
                og[:, h, :], gT[:, h, :], 1.0, poT, ALU.add, ALU.mult)
            qadd_flush(st_)
            if h + 1 < H:
                proj_g(st_, h + 1)

            if h % 2 == 1:
                if psums is None:
                    psums = sums_emit()
                i = h // 2
                # fast reciprocal written straight into an f32r tile (the
                # DVE rounds on write, satisfying the f32r-matmul input
                # rule without a separate copy)
                from concourse.dve_ops import (RECIP_APPROX_FAST_CONSTS,
                                               RECIPROCAL_APPROX_FAST)
                rb2 = bpool.tile([2, 512], F16, tag=f"rb2_{i}",
                                 name=f"rb2_{b}_{i}_{rep}")
                c_ = RECIP_APPROX_FAST_CONSTS
                with nc.allow_low_precision(reason="softmax recip"):
                    nc.vector._custom_dve(
                        RECIPROCAL_APPROX_FAST, out=rb2, in0=psums,
                        s0=c_["s0"], s1=c_["s1"], imm2=c_["imm2"])
                if DEBUG and b == 0 and i == 0:
                    pass  # d_sums debug dropped (rb2 is fp16 now)
                st_["pending"] = (i, rb2)
            st_["attnP"] = attnT

            if DEBUG and b == 0 and h == 0:
                nc.sync.dma_start(t["d_attnT"][:], attnT[:])
                po_sb = bpool.tile([128, 512], F32, tag="po_sb")
                nc.vector.tensor_copy(po_sb, poT)
                nc.sync.dma_start(t["d_oT"][:], po_sb)

        def outproj(st_):
            b, og = st_["b"], st_["og"]
            if DEBUG and b == 0:
                for nm, tl in [("d_qT", st_["qT"]), ("d_gT", st_["gT"]),
                               ("d_kT", st_["kT"]), ("d_vv", st_["vv"]),
                               ("d_og", og)]:
                    nc.sync.dma_start(t[nm][:], tl[:])
            # chunks 0-5 first so they hide the last pair's reciprocal;
            # the last pair is then normalized and folded in per-head, and
            # y is copied/DMA'd in halves to overlap the final transfer
            i, rb = st_["pending"]
            st_["pending"] = None
            pf = psA.tile([128, 512], F32, tag="psA")
            for c in range(6):
                nc.tensor.matmul(pf, wo[:, c, :], og[:, c, :],
                                 start=(c == 0), stop=False)
            prb = ps3.tile([128, 2, 512], F32, tag="ps3")
            for j, c in enumerate((6, 7)):
                nc.tensor.matmul(prb[:, j, :], sel2[:, j, :], rb,
                                 start=True, stop=True)
                nc.vector.tensor_tensor(og[:, c, :], og[:, c, :],
                                        prb[:, j, :], ALU.mult)
                nc.tensor.matmul(pf, wo[:, c, :], og[:, c, :],
                                 start=False, stop=(c == 7))
            y = ypool.tile([128, 512], F16, tag="y")
            nc.scalar.copy(y[:, 0:256], pf[:, 0:256])
            nc.sync.dma_start(t["yT"][b][:, 0:256], y[:, 0:256])
            nc.vector.tensor_copy(y[:, 256:512], pf[:, 256:512])
            nc.scalar.dma_start(t["yT"][b][:, 256:512], y[:, 256:512])

        # software pipeline: the two batches are interleaved head-by-head
        # with a 2-head stagger, so every intra-head latency chain (exp ->
        # AV, gating -> PSUM reuse, reciprocal -> normalize) is covered by
        # the other batch's independent matmuls
        st0 = prologue(0, 0, x=x0)
        for rep in range(CFG["nrep"]):
            head(st0, 0)
            head(st0, 1)
            st1 = prologue(1, rep)
            head(st0, 2)
            for k in range(3, H):
                head(st0, k)
                head(st1, k - 3)
            nxt = None
            if rep + 1 < CFG["nrep"]:
                nxt = prologue(0, rep + 1)
            outproj(st0)
            for k in range(H - 3, H):
                head(st1, k)
            outproj(st1)
            if nxt is not None:
                st0 = nxt


# ---------------------------------------------------------------- build
_CACHE = {}


def _get_nc():
    key = tuple(sorted(CFG.items()))
    if _CACHE.get("key") == key:
        return _CACHE["nc"], _CACHE["t"]
    nc = bacc.Bacc(None, target_bir_lowering=False)
    t = dict(
        xT=nc.dram_tensor("xT", [B, 128, SL], F16, kind="ExternalInput"),
        biasT=nc.dram_tensor("biasT", [B, 128, NBLK, 128], F16,
                             kind="ExternalInput"),
        wqT=nc.dram_tensor("wqT", [128, DI], F16, kind="ExternalInput"),
        wkT=nc.dram_tensor("wkT", [128, DI], F16, kind="ExternalInput"),
        wvT=nc.dram_tensor("wvT", [128, DI], F16, kind="ExternalInput"),
        wgoT=nc.dram_tensor("wgoT", [128, DI + 1024], F16,
                            kind="ExternalInput"),
        cblob=nc.dram_tensor("cblob", [128, 340], F32,
                             kind="ExternalInput"),
        yT=nc.dram_tensor("yT", [B, 128, SC], F16, kind="ExternalOutput"),
    )
    if DEBUG:
        for nm, shp, dt_ in [("d_qT", [128, H, SC], F16),
                             ("d_gT", [128, H, SC], F32),
                             ("d_kT", [128, H, SL], F16),
                             ("d_vv", [128, NJ, DI], F16),
                             ("d_og", [128, H, SC], F16),
                             ("d_attnT", [128, NBLK, 128], F16),
                             ("d_oT", [128, 512], F32),
                             ("d_sums", [2, 512], F32)]:
            t[nm] = nc.dram_tensor(nm, shp, dt_, kind="ExternalOutput")
    _build_device(nc, t)
    nc.compile()
    _CACHE["nc"], _CACHE["t"], _CACHE["key"] = nc, t, key
    return nc, t


# ---------------------------------------------------------------- host
def _prep_shared(Wq, bq, Wkv, Wg, bg, Wo):
    scale = DH ** -0.5
    wqT = np.ascontiguousarray((Wq * scale).T, np.float16)          # [128,1024]
    wkT = np.ascontiguousarray(Wkv[:DI].T, np.float16)
    wvT = np.ascontiguousarray(Wkv[DI:].T, np.float16)
    wgT = np.ascontiguousarray(Wg.T, np.float16)
    # gating uses (tanh(0.5 z)+1) and final matmul absorbs the 0.5
    woT = np.ascontiguousarray(
        (0.5 * Wo).T.reshape(8, 128, 128).transpose(1, 0, 2), np.float16
    )                                                                # [128,8,128]
    bq8 = np.ascontiguousarray((bq * scale).reshape(8, 128).T, np.float32)
    bg8 = np.ascontiguousarray((bg * 0.5).reshape(8, 128).T, np.float32)
    ident = np.eye(128, dtype=np.float16)
    ones2 = np.zeros((128, 2, 2), np.float16)
    ones2[:, 0, 0] = 1.0
    ones2[:, 1, 1] = 1.0
    sel2 = np.zeros((2, 2, 128), np.float32)
    sel2[0, 0, :] = 1.0
    sel2[1, 1, :] = 1.0
    cblob = np.zeros((128, 340), np.float32)
    cblob[:, 0:64] = ident.view(np.float32).reshape(128, 64)
    cblob[:, 64:66] = ones2.reshape(128, 4).view(np.float32)
    cblob[:, 66:74] = bq8
    cblob[:, 74:82] = bg8
    cblob[0:2, 84:212] = sel2.reshape(2, 256).astype(np.float16) \
        .view(np.float32)
    wgoT = np.concatenate(
        [wgT, woT.reshape(128, 1024)], axis=1)             # [128,2048] f16
    return dict(wqT=wqT, wkT=wkT, wvT=wvT, wgoT=wgoT, cblob=cblob)


def _prep_core(c, seq, attn_bias):
    lo = c * SC - W
    hi = c * SC + SC + W
    xs = np.zeros((B, SL, D), np.float16)
    a, bnd = max(lo, 0), min(hi, S)
    xs[:, a - lo : bnd - lo, :] = seq[:, a:bnd, :].astype(np.float16)
    xT = np.ascontiguousarray(xs.transpose(0, 2, 1))                 # [B,128,768]

    br = attn_bias.reshape(B, NWIN, W, NWIN, W)
    biasT = np.full((B, 128, NBLK, 128), NEG, np.float16)
    for J in range(-1, NW + 1):
        gk = NW * c + J                     # global key window
        if not (0 <= gk < NWIN):
            continue
        for wi in range(NK[J]):
            w = WSTART[J] + wi
            gq = NW * c + w
            blk = br[:, gq, :, gk, :]       # [B, q(128), k(128)]
            biasT[:, :, BSTART[J] + wi, :] = \
                blk.transpose(0, 2, 1).astype(np.float16)
    return xT, biasT


def kernel(seq, mask, attn_bias, Wq, bq, Wkv, Wg, bg, Wo):
    from concourse.bass_utils import run_bass_kernel_spmd

    nc, _ = _get_nc()
    seq = np.asarray(seq, np.float32)
    attn_bias = np.asarray(attn_bias, np.float32)
    shared = _prep_shared(
        np.asarray(Wq, np.float32), np.asarray(bq, np.float32),
        np.asarray(Wkv, np.float32), np.asarray(Wg, np.float32),
        np.asarray(bg, np.float32), np.asarray(Wo, np.float32),
    )
    in_maps = []
    for c in range(NCORES):
        xT, biasT = _prep_core(c, seq, attn_bias)
        in_maps.append(dict(xT=xT, biasT=biasT, **shared))

    res = run_bass_kernel_spmd(nc, in_maps, core_ids=list(range(NCORES)))
    y = np.empty((B, S, D), np.float32)
    for c in range(NCORES):
        yT = res.results[c]["yT"]                                    # [B,128,512]
        y[:, c * SC : (c + 1) * SC, :] = yT.transpose(0, 2, 1).astype(np.float32)
    return y
